# revision 28
# baseline (speedup 1.0000x reference)
"""Trainium2 Bass kernel for LoFTR-style linear attention (nn_AttentionLayer).

Data-parallel over B=1024 across 8 NeuronCores (128 batches/core, no
collectives).  Q/K projections run in fp8(e4m3) DoubleRow mode (2 contraction
planes per pass = 2x bf16 throughput); V projection and the attention einsums
stay bf16.  fp8 on Q/K is accuracy-safe because the Z-normalization cancels
their quantization errors (measured ~3.7e-3 vs 2e-2 budget); fp8 on V is not.

Key tricks:
  - xg scaled by SX=16 and weights by SW=256 before e4m3 quantization.
  - Q and K biases (+1) enter the fp8 matmuls as two constant contraction
    rows (value 32) against w6=e4m3(128*(b+1)) and its e4m3 residual w7, so
    both PSUMs hold 4096*(z+1) and no per-partition/free-dim bias is needed.
  - Scale-domain phi: phi(z) = elu(z)+1 = max(z+1, min(exp(z), 1)).  With
    e' = exp(z + ln 4096) = 4096 exp(z) (ACT Exp, constant bias ln(4096)-1),
    a single DVE stt computes 4096*phi = (e' min 4096) max PSUM directly --
    no separate min op, no Identity pass.  The 4096 factor cancels in
    out = U/den, so phi can stay scaled through the attention matmuls.
  - Q PSUM is a (128,2,512) two-bank tile per m-pair: one Exp and one stt
    cover two m-tiles (per-op fixed overhead is ~370ns, so fewer+bigger ops).
  - V bias is applied on host: out = U/den + bv exactly (the [V|1] ones
    column makes the bias term den*bv).  V' staging is then a pure copy.
  - KV' staged into a single (128,4,130) tile with statically-zeroed dual
    quadrants; U runs one N=130 matmul per head-pair at full K=128.
  - No on-chip normalization: [U | den] is copied PSUM->SBUF and DMA'd out;
    the host divides (GPSIMD cannot run elementwise ops on this target, so
    ACT/DVE PSUM-drain is the scarce resource).
  - Software pipelining: KV/U of batch b execute between the K/V
    projections of batch b+1, hiding the phi dependency chain and keeping
    the PE dense (p-state!).
  - Inputs are repacked host-side into per-block contiguous slabs: 2 input
    DMAs per block, 1 output DMA per batch (DMA_DIRECT2D runs ~650ns on the
    Sync engine; it was a co-bottleneck at 18 DMAs/block).
"""

import numpy as np
import ml_dtypes

NCORES = 8
B, L, HID, GUID, H, D = 1024, 128, 512, 256, 8, 64
KIN = HID + GUID          # 768
BSH = B // NCORES         # 128 batches per core
TOK = BSH * L             # 16384 tokens per core
BLK = 512                 # tokens per block (4 batches)
NBLK = TOK // BLK         # 32 blocks
SX = 16.0                 # fp8 activation scale
SW = 256.0                # fp8 weight scale
DS = 1.0 / (SX * SW)      # 2^-12 descale
EPS = 1e-6

_CACHE = {}


def _build(nblocks):
    from contextlib import ExitStack
    import concourse.bass as bass
    import concourse.mybir as mybir
    import concourse.tile as tile
    from concourse import bacc

    f32 = mybir.dt.float32
    bf16 = mybir.dt.bfloat16
    fp8 = mybir.dt.float8e4
    AF = mybir.ActivationFunctionType
    OP = mybir.AluOpType
    DR = mybir.MatmulPerfMode.DoubleRow

    nc = bacc.Bacc("TRN2", target_bir_lowering=False, debug=False,
                   num_devices=NCORES)

    xg_d = nc.dram_tensor("xg8B", (nblocks, 128, 6, 512), fp8,
                          kind="ExternalInput").ap()
    xv_d = nc.dram_tensor("xvB", (nblocks, 128, 4, 512), bf16,
                          kind="ExternalInput").ap()
    wq_d = nc.dram_tensor("wq8", (128, 6, 512), fp8, kind="ExternalInput").ap()
    wk_d = nc.dram_tensor("wk8", (128, 8, 512), fp8, kind="ExternalInput").ap()
    wv_d = nc.dram_tensor("wv", (128, 4, 512), bf16, kind="ExternalInput").ap()
    qb_d = nc.dram_tensor("qb", (128, 4, 2), f32, kind="ExternalInput").ap()
    u_d = nc.dram_tensor("u", (nblocks * BLK, 520), bf16,
                         kind="ExternalOutput").ap()

    with tile.TileContext(nc) as tc, ExitStack() as ctx:
        consts = ctx.enter_context(tc.tile_pool(name="consts", bufs=1))
        xv_pool = ctx.enter_context(tc.tile_pool(name="xv", bufs=3))
        qphi_pool = ctx.enter_context(tc.tile_pool(name="qphi", bufs=2))
        tmp_pool = ctx.enter_context(tc.tile_pool(name="tmp", bufs=6))
        kphi_pool = ctx.enter_context(tc.tile_pool(name="kphi", bufs=2))
        out_pool = ctx.enter_context(tc.tile_pool(name="outp", bufs=4))
        psq_pool = ctx.enter_context(tc.tile_pool(name="psq", bufs=1, space="PSUM"))
        pskv_pool = ctx.enter_context(tc.tile_pool(name="pskv", bufs=3, space="PSUM"))
        psatt_pool = ctx.enter_context(tc.tile_pool(name="psatt", bufs=1, space="PSUM"))
        psu_pool = ctx.enter_context(tc.tile_pool(name="psu", bufs=1, space="PSUM"))

        wq_t = consts.tile([128, 6, 512], fp8)
        wk_t = consts.tile([128, 8, 512], fp8)
        wv_t = consts.tile([128, 4, 512], bf16)
        qb_t = consts.tile([128, 4, 2], f32)
        # Exp bias: exp(z + ln4096) = 4096 exp(z); K PSUM holds 4096(z+1)
        eb_t = consts.tile([128, 1], f32)
        nc.vector.memset(eb_t[:], float(np.log(4096.0) - 1.0))
        nc.sync.dma_start(wq_t[:], wq_d[:])
        nc.sync.dma_start(wk_t[:], wk_d[:])
        nc.sync.dma_start(wv_t[:], wv_d[:])
        nc.sync.dma_start(qb_t[:], qb_d[:])

        # xg fp8 tiles: manual 3-buffer rotation; planes 6/7 are the constant
        # bias rows for the Q/K projections (row 0 = 32, rest 0), set once.
        xg8 = [consts.tile([128, 8, 512], fp8, name=f"xg8_{i}") for i in range(3)]
        for i in range(3):
            nc.vector.memset(xg8[i][:, 6:8, :], 0.0)
            nc.vector.memset(xg8[i][0:1, 6:8, :], 32.0)

        # V' staging tiles with static ones column (the Ksum column of KV'),
        # and KV' tiles with statically-zeroed dual quadrants so U can run
        # full K=128 against [E|O] packed columns.
        vp = [consts.tile([128, 8, 65], bf16, name=f"vp{i}") for i in range(2)]
        kv2 = [consts.tile([128, 4, 130], bf16, name=f"kv2_{i}") for i in range(2)]
        for i in range(2):
            nc.vector.memset(vp[i][:, :, 64:65], 1.0)
            nc.vector.memset(kv2[i][64:128, :, 0:65], 0.0)
            nc.vector.memset(kv2[i][0:64, :, 65:130], 0.0)

        # pipeline state: (b, bs, kphi_t, qphi_t) of the batch whose KV/U
        # stage has not been emitted yet
        pend = [None]

        def emit_kv(pv):
            b, _, kphi_t, _ = pv
            ps_kv_full = psatt_pool.tile([128, 512], f32, tag="psatt",
                                         name="ps_kv")
            ps_kv = ps_kv_full[:, :260]
            vp_t = vp[b % 2]
            for p in range(4):
                nc.tensor.matmul(
                    ps_kv[0:64, p * 65:(p + 1) * 65],
                    kphi_t[:, p * 128:p * 128 + 64],
                    vp_t[:, 2 * p, :],
                    start=True, stop=True, tile_position=(0, 0))
                nc.tensor.matmul(
                    ps_kv[64:128, p * 65:(p + 1) * 65],
                    kphi_t[:, p * 128 + 64:(p + 1) * 128],
                    vp_t[:, 2 * p + 1, :],
                    start=True, stop=True, tile_position=(0, 64))
            kv2_t = kv2[b % 2]
            # both copies on ACT: DVE is the hotter engine
            nc.scalar.copy(
                kv2_t[0:64, :, 0:65],
                ps_kv[0:64, :].rearrange("p (c j) -> p c j", j=65))
            nc.scalar.copy(
                kv2_t[64:128, :, 65:130],
                ps_kv[64:128, :].rearrange("p (c j) -> p c j", j=65))

        def emit_u(pv):
            b, bs, _, qphi_t = pv
            kv2_t = kv2[b % 2]
            uo = out_pool.tile([128, 2, 260], bf16, tag="outp")
            # both halves in one 2-bank PSUM tile -> one fused copy
            ps_u2 = psu_pool.tile([128, 2, 512], f32, tag="psu", name="ps_u")
            for half in range(2):
                for pp in range(2):
                    p = half * 2 + pp
                    nc.tensor.matmul(
                        ps_u2[:, half, pp * 130:(pp + 1) * 130],
                        qphi_t[:, p, bs],
                        kv2_t[:, p, :],
                        start=True, stop=True)
            # alternate the PSUM->SBUF copy between ACT and DVE per batch
            if b % 2 == 0:
                nc.scalar.copy(uo[:], ps_u2[:, :, 0:260])
            else:
                nc.vector.tensor_copy(uo[:], ps_u2[:, :, 0:260])
            nc.sync.dma_start(u_d[b * 128:(b + 1) * 128, :],
                              uo[:].rearrange("p h j -> p (h j)"))

        def emit_qpair(xg_t, qphi_t, q):
            # two m-tiles (2q, 2q+1) share one 2-bank PSUM tile.  Q is
            # feature-on-partition, so bq rides per-partition operands: no
            # pad matmul needed (PSUM = 4096*z_raw), Exp bias carries
            # bq + ln4096, the stt scalar adds 4096*(bq+1).
            ps2 = psq_pool.tile([128, 2, 512], f32, tag="psq")
            for mi in range(2):
                m = 2 * q + mi
                for kp in range(3):
                    nc.tensor.matmul(
                        ps2[:, mi, :],
                        wq_t[:, 2 * kp:2 * kp + 2, m * 128:(m + 1) * 128],
                        xg_t[:, 2 * kp:2 * kp + 2, :],
                        start=(kp == 0), stop=(kp == 2), perf_mode=DR,
                    )
            for mi in range(2):
                m = 2 * q + mi
                e = tmp_pool.tile([128, 512], bf16, tag="tmp")
                nc.scalar.activation(e[:], ps2[:, mi, :], AF.Exp,
                                     bias=qb_t[:, m, 0:1], scale=DS)
                t = tmp_pool.tile([128, 512], bf16, tag="tmp")
                nc.vector.tensor_scalar_min(t[:], e[:], 4096.0)
                nc.vector.scalar_tensor_tensor(
                    qphi_t[:, m, :], ps2[:, mi, :], qb_t[:, m, 1:2], t[:],
                    OP.add, OP.max)

        for j in range(nblocks):
            xg_t = xg8[j % 3]
            nc.sync.dma_start(xg_t[:, 0:6, :], xg_d[j])
            xv_t = xv_pool.tile([128, 4, 512], bf16, tag="xv")
            nc.sync.dma_start(xv_t[:], xv_d[j])

            qphi_t = qphi_pool.tile([128, 4, 512], bf16, tag="qphi")
            emit_qpair(xg_t, qphi_t, 0)

            for bi in range(4):
                b = j * 4 + bi
                bs = slice(bi * 128, (bi + 1) * 128)

                # ---- K projection (token-on-partition, fp8 DoubleRow);
                # pair (6,7) carries the (bk+1) bias rows ----
                ps_k = pskv_pool.tile([128, 512], f32, tag="pskv")
                for kp in range(4):
                    nc.tensor.matmul(
                        ps_k[:],
                        xg_t[:, 2 * kp:2 * kp + 2, bs],
                        wk_t[:, 2 * kp:2 * kp + 2, :],
                        start=(kp == 0), stop=(kp == 3), perf_mode=DR,
                    )

                # KV of the previous batch runs here (kphi/vp are ready by
                # now); its U follows after this batch's V projection
                if pend[0] is not None:
                    emit_kv(pend[0])

                # ---- phi(K): e' = 4096 exp(z), then one stt ----
                e = tmp_pool.tile([128, 512], bf16, tag="tmp")
                nc.scalar.activation(e[:], ps_k[:], AF.Exp,
                                     bias=eb_t[:], scale=DS)
                kphi_t = kphi_pool.tile([128, 512], bf16, tag="kphi")
                nc.vector.scalar_tensor_tensor(
                    kphi_t[:], e[:], 4096.0, ps_k[:], OP.min, OP.max)

                # ---- V projection (bf16, no bias: bv is added on host) ----
                ps_v = pskv_pool.tile([128, 512], f32, tag="pskv")
                for k in range(4):
                    nc.tensor.matmul(ps_v[:], xv_t[:, k, bs], wv_t[:, k, :],
                                     start=(k == 0), stop=(k == 3))

                if pend[0] is not None:
                    emit_u(pend[0])

                # second Q m-pair goes mid-block so its PSUM (bufs=1) has
                # time to drain and the PE stays dense
                if bi == 0:
                    emit_qpair(xg_t, qphi_t, 1)

                # ---- V' = [V | 1] per head (plain PSUM->SBUF copy) ----
                vp_t = vp[b % 2]
                nc.vector.tensor_copy(
                    vp_t[:, :, 0:64],
                    ps_v[:].rearrange("p (h d) -> p h d", d=64))

                pend[0] = (b, bs, kphi_t, qphi_t)

        emit_kv(pend[0])
        emit_u(pend[0])

    nc.compile()
    return nc


def _get_nc(nblocks=NBLK):
    if nblocks not in _CACHE:
        _CACHE[nblocks] = _build(nblocks)
    return _CACHE[nblocks]


def _pad_rows(bias):
    """e4m3 bias rows: 32*w6 + 32*w7 == 4096*(bias+1) with residual split."""
    f8 = ml_dtypes.float8_e4m3
    w6 = (128.0 * (bias + 1.0)).astype(f8)
    w7 = ((4096.0 * (bias + 1.0) - 32.0 * w6.astype(np.float32)) / 32.0
          ).astype(f8)
    return w6, w7


def _prep_shared(Wq, bq, Wk, bk, Wv, bv):
    bf = ml_dtypes.bfloat16
    f8 = ml_dtypes.float8_e4m3
    wq8 = np.ascontiguousarray(
        (Wq.reshape(6, 128, 512).transpose(1, 0, 2)) * SW).astype(f8)
    wk8 = np.zeros((128, 8, 512), f8)
    wk8[:, 0:6, :] = ((Wk.reshape(6, 128, 512).transpose(1, 0, 2)) * SW
                      ).astype(f8)
    wk8[0, 6, :], wk8[0, 7, :] = _pad_rows(bk)
    wv = np.ascontiguousarray(
        Wv.reshape(4, 128, 512).transpose(1, 0, 2)).astype(bf)
    qb = np.ascontiguousarray(np.stack(
        [bq.reshape(4, 128).T + float(np.log(4096.0)),
         4096.0 * (bq.reshape(4, 128).T + 1.0)],
        axis=-1)).astype(np.float32)
    return wq8, wk8, wv, qb


def _prep_core(x_c, g_c):
    bf = ml_dtypes.bfloat16
    f8 = ml_dtypes.float8_e4m3
    xs = np.asarray(x_c).reshape(TOK, HID)
    gs = np.asarray(g_c).reshape(TOK, GUID)
    xg = np.concatenate([xs, gs], axis=1)
    # per-block contiguous slabs: [block, partition, chunk, token]
    xg8B = np.ascontiguousarray(
        (xg * SX).reshape(NBLK, BLK, 6, 128).transpose(0, 3, 2, 1)).astype(f8)
    xvB = np.ascontiguousarray(
        xs.reshape(NBLK, BLK, 4, 128).transpose(0, 3, 2, 1)).astype(bf)
    return xg8B, xvB


def _finish(u, bv):
    # u: (TOK, 520) bf16 = per token 2 halves x [4 heads x (64 out | den)]
    u = np.asarray(u).astype(np.float32).reshape(TOK, 8, 65)
    return u[:, :, 0:64] / (u[:, :, 64:65] + EPS) + bv.reshape(1, 8, 64)


def kernel(x, guidance, Wq, bq, Wk, bk, Wv, bv):
    from concourse.bass_utils import run_bass_kernel_spmd

    x = np.asarray(x, dtype=np.float32)
    guidance = np.asarray(guidance, dtype=np.float32)
    Wq = np.asarray(Wq, dtype=np.float32)
    bq = np.asarray(bq, dtype=np.float32)
    Wk = np.asarray(Wk, dtype=np.float32)
    bk = np.asarray(bk, dtype=np.float32)
    Wv = np.asarray(Wv, dtype=np.float32)
    bv = np.asarray(bv, dtype=np.float32)

    nc = _get_nc()
    wq8, wk8, wv, qb = _prep_shared(Wq, bq, Wk, bk, Wv, bv)

    in_maps = []
    for c in range(NCORES):
        xg8B, xvB = _prep_core(x[c * BSH:(c + 1) * BSH],
                               guidance[c * BSH:(c + 1) * BSH])
        in_maps.append({"xg8B": xg8B, "xvB": xvB, "wq8": wq8, "wk8": wk8,
                        "wv": wv, "qb": qb})

    res = run_bass_kernel_spmd(nc, in_maps, core_ids=list(range(NCORES)))
    outs = [_finish(r["u"], bv) for r in res.results]
    return np.concatenate(outs, axis=0).reshape(B, L, H * D).astype(np.float32)
